# revision 1
# baseline (speedup 1.0000x reference)
"""Bipartite GNN edge decoder on 8 Trainium2 NeuronCores.

Computation (per edge e with endpoints row[e], col[e]):
    z = [z_src[row[e]], z_dst[col[e]]]          # [256]
    h = relu(z @ W1.T + b1)                     # [128]
    out[e] = sigmoid(h @ W2.T + b2)             # scalar

Distribution strategy (a blend of both options in the sharding hint):
the node tables are range-partitioned into 4 src-windows x 2 dst-windows
of 25000 rows each, and every edge is assigned to the core owning its
(src-window, dst-window) pair — data-parallel over edges with each core
holding only its two 12.8 MB table windows and window-local indices.
For uniformly random edges the 8 buckets are balanced to ~0.5%.

Per core, edges are processed in 2048-edge tiles: two `dma_gather`
custom DMA instructions (on separate SWDGE queues, so their descriptor
generation runs on different GPSIMD core pairs) gather the endpoint
rows (512 B each) into SBUF with edges on partitions. PE transposes
flip features onto partitions, two accumulating matmuls apply the two
halves of W1, ACT fuses bias+ReLU, per-128-edge matmuls against the W2
column put logits back with edges on partitions, ACT fuses b2+sigmoid,
and one contiguous DMA stores each tile's results. The host applies
the inverse edge permutation to assemble the full output.
"""
import os
import numpy as np

import concourse.bass as bass
import concourse.bacc as bacc
import concourse.mybir as mybir
from concourse.tile import TileContext
from concourse.masks import make_identity
from concourse.bass_utils import run_bass_kernel_spmd

# Problem shapes (fixed by the task)
N_SRC, N_DST, E, H = 100000, 50000, 1000000, 128
N_CORES = 8

P = 128
KG = 16                      # 128-row chunks per gather tile
GT = P * KG                  # 2048 edges per gather tile
S = GT // 16                 # idx free dim (16-partition wrap)
ST_BLKS = 4                  # 128-edge blocks per matmul supertile
WIN_SRC = N_SRC // 4         # 25000 rows per src window (4 windows)
WIN_DST = N_DST // 2         # 25000 rows per dst window (2 windows)

_cache = {}
_last_results = None         # test harness reads exec_time_ns from here


def _build_program(n_tiles):
    fp32 = mybir.dt.float32
    int32 = mybir.dt.int32
    nc = bacc.Bacc(trn_type="TRN2")

    zsrc_d = nc.dram_tensor("z_src_win", [WIN_SRC, H], fp32, kind="ExternalInput")
    zdst_d = nc.dram_tensor("z_dst_win", [WIN_DST, H], fp32, kind="ExternalInput")
    risrc_d = nc.dram_tensor("idx_src", [n_tiles * P, KG], int32, kind="ExternalInput")
    ridst_d = nc.dram_tensor("idx_dst", [n_tiles * P, KG], int32, kind="ExternalInput")
    W1_d = nc.dram_tensor("W1", [H, 2 * H], fp32, kind="ExternalInput")
    b1_d = nc.dram_tensor("b1", [H], fp32, kind="ExternalInput")
    W2_d = nc.dram_tensor("W2", [1, H], fp32, kind="ExternalInput")
    b2_d = nc.dram_tensor("b2", [1], fp32, kind="ExternalInput")
    out_d = nc.dram_tensor("out", [n_tiles * GT], fp32, kind="ExternalOutput")

    with TileContext(nc) as tc:
        with (
            tc.tile_pool(name="const", bufs=1) as cpool,
            tc.tile_pool(name="sbuf", bufs=2) as spool,
            tc.tile_pool(name="psum", bufs=2, space="PSUM") as ppool,
            tc.tile_pool(name="psum2", bufs=2, space="PSUM") as ppool2,
        ):
            # ---- one-time prep ----
            ident = cpool.tile([P, P], fp32)
            make_identity(nc, ident[:])

            w1_s = cpool.tile([P, 2 * H], fp32)            # [hf, f0|f1]
            nc.sync.dma_start(out=w1_s[:], in_=W1_d[:])
            w1aT = cpool.tile([P, P], fp32)                # [f, hf]
            w1bT = cpool.tile([P, P], fp32)
            for i, dstT in enumerate((w1aT, w1bT)):
                tp = ppool2.tile([P, P], fp32, tag="logit")
                nc.tensor.transpose(tp[:], w1_s[:, i * H:(i + 1) * H], ident[:])
                nc.vector.tensor_copy(dstT[:], tp[:])

            b1col = cpool.tile([P, 1], fp32)
            nc.sync.dma_start(out=b1col[:], in_=b1_d[:, None])
            w2col = cpool.tile([P, 1], fp32)
            nc.sync.dma_start(out=w2col[:], in_=W2_d[0, :, None])

            # broadcast the b2 scalar to all partitions via a ones-matmul
            b2_s = cpool.tile([1, 1], fp32)
            nc.sync.dma_start(out=b2_s[:], in_=b2_d[:, None])
            ones_s = cpool.tile([1, P], fp32)
            nc.gpsimd.memset(ones_s[:], 1.0)
            b2p = ppool2.tile([P, 1], fp32, tag="logit")
            nc.tensor.matmul(b2p[:], lhsT=ones_s[:], rhs=b2_s[:], start=True, stop=True)
            b2col = cpool.tile([P, 1], fp32)
            nc.vector.tensor_copy(b2col[:], b2p[:])

            # ---- edge tiles ----
            for t in range(n_tiles):
                base = t * GT
                idx_s = spool.tile([P, KG], int32, tag="idxs")
                idx_d = spool.tile([P, KG], int32, tag="idxd")
                nc.sync.dma_start(out=idx_s[:], in_=risrc_d[t * P:(t + 1) * P, :])
                nc.sync.dma_start(out=idx_d[:], in_=ridst_d[t * P:(t + 1) * P, :])

                zs = spool.tile([P, KG * H], fp32, tag="zs")
                zd = spool.tile([P, KG * H], fp32, tag="zd")
                for k in range(KG):
                    nc.gpsimd.indirect_dma_start(
                        out=zs[:, k * H:(k + 1) * H], out_offset=None, in_=zsrc_d[:],
                        in_offset=bass.IndirectOffsetOnAxis(ap=idx_s[:, k:k + 1], axis=0))
                    nc.gpsimd.indirect_dma_start(
                        out=zd[:, k * H:(k + 1) * H], out_offset=None, in_=zdst_d[:],
                        in_offset=bass.IndirectOffsetOnAxis(ap=idx_d[:, k:k + 1], axis=0))

                logit_ps = ppool2.tile([P, KG], fp32, tag="logit")
                sig_s = spool.tile([P, KG], fp32, tag="sig")

                for s in range(KG // ST_BLKS):
                    nb = ST_BLKS
                    zsT_ps = ppool.tile([P, nb * P], fp32, tag="zsT")
                    zdT_ps = ppool.tile([P, nb * P], fp32, tag="zdT")
                    for b in range(nb):
                        k = s * nb + b
                        nc.tensor.transpose(
                            zsT_ps[:, b * P:(b + 1) * P], zs[:, k * H:(k + 1) * H], ident[:])
                        nc.tensor.transpose(
                            zdT_ps[:, b * P:(b + 1) * P], zd[:, k * H:(k + 1) * H], ident[:])
                    zsT_s = spool.tile([P, nb * P], fp32, tag="zsTs")
                    zdT_s = spool.tile([P, nb * P], fp32, tag="zdTs")
                    nc.vector.tensor_copy(zsT_s[:], zsT_ps[:])   # DVE
                    nc.scalar.copy(zdT_s[:], zdT_ps[:])          # ACT

                    hT_ps = ppool.tile([P, nb * P], fp32, tag="hT")
                    nc.tensor.matmul(hT_ps[:], lhsT=w1aT[:], rhs=zsT_s[:],
                                     start=True, stop=False)
                    nc.tensor.matmul(hT_ps[:], lhsT=w1bT[:], rhs=zdT_s[:],
                                     start=False, stop=True)

                    hT_s = spool.tile([P, nb * P], fp32, tag="hTs")
                    nc.scalar.activation(
                        hT_s[:], hT_ps[:], mybir.ActivationFunctionType.Relu,
                        bias=b1col[:, 0:1])

                    for b in range(nb):
                        k = s * nb + b
                        nc.tensor.matmul(
                            logit_ps[:, k:k + 1], lhsT=hT_s[:, b * P:(b + 1) * P],
                            rhs=w2col[:], start=True, stop=True)

                nc.scalar.activation(
                    sig_s[:], logit_ps[:], mybir.ActivationFunctionType.Sigmoid,
                    bias=b2col[:, 0:1])
                nc.sync.dma_start(
                    out=out_d[base:base + GT].rearrange("(p k) -> p k", p=P),
                    in_=sig_s[:])
    nc.compile()
    return nc


def _wrap_idx(idx, n_tiles):
    """[n_tiles*GT] int32 -> [n_tiles*P, KG]: edge slot (t, p, k) holds the
    window-local index of edge t*GT + p*KG + k (p-major tile layout)."""
    return np.ascontiguousarray(idx.reshape(n_tiles * P, KG))


def _run(inputs, trace=False):
    global _last_results

    z_src = np.ascontiguousarray(np.asarray(inputs["z_src"], dtype=np.float32))
    z_dst = np.ascontiguousarray(np.asarray(inputs["z_dst"], dtype=np.float32))
    eli = np.asarray(inputs["edge_label_index"])
    row = np.ascontiguousarray(eli[0]).astype(np.int64)
    col = np.ascontiguousarray(eli[1]).astype(np.int64)
    W1 = np.ascontiguousarray(np.asarray(inputs["W1"], dtype=np.float32))
    b1 = np.ascontiguousarray(np.asarray(inputs["b1"], dtype=np.float32))
    W2 = np.ascontiguousarray(np.asarray(inputs["W2"], dtype=np.float32))
    b2 = np.ascontiguousarray(np.asarray(inputs["b2"], dtype=np.float32))

    # bucket edges by (src window, dst window) -> owning core
    ws = row // WIN_SRC
    wd = col // WIN_DST
    bucket = (ws * 2 + wd).astype(np.int64)
    perm = np.argsort(bucket, kind="stable")
    counts = np.bincount(bucket, minlength=N_CORES)
    starts = np.concatenate([[0], np.cumsum(counts)])
    n_tiles = max(1, int(-(-counts.max() // GT)))
    cap = n_tiles * GT

    key = n_tiles
    if _cache.get("key") != key:
        _cache["nc"] = _build_program(n_tiles)
        _cache["key"] = key
    nc = _cache["nc"]

    in_maps = []
    sels = []
    for c in range(N_CORES):
        sel = perm[starts[c]:starts[c + 1]]
        sels.append(sel)
        r16 = np.zeros(cap, dtype=np.int32)
        c16 = np.zeros(cap, dtype=np.int32)
        r16[:len(sel)] = (row[sel] - (c // 2) * WIN_SRC).astype(np.int32)
        c16[:len(sel)] = (col[sel] - (c % 2) * WIN_DST).astype(np.int32)
        in_maps.append({
            "z_src_win": z_src[(c // 2) * WIN_SRC:(c // 2 + 1) * WIN_SRC],
            "z_dst_win": z_dst[(c % 2) * WIN_DST:(c % 2 + 1) * WIN_DST],
            "idx_src": _wrap_idx(r16, n_tiles),
            "idx_dst": _wrap_idx(c16, n_tiles),
            "W1": W1, "b1": b1, "W2": W2, "b2": b2,
        })

    try:
        res = run_bass_kernel_spmd(nc, in_maps, core_ids=list(range(N_CORES)),
                                   trace=trace)
    except ImportError:
        # BASS_TRACE set but the NTFF profile hook isn't available in this
        # environment — rerun untraced.
        os.environ.pop("BASS_TRACE", None)
        res = run_bass_kernel_spmd(nc, in_maps, core_ids=list(range(N_CORES)),
                                   trace=False)
    _last_results = res

    out = np.empty(E, dtype=np.float32)
    for c in range(N_CORES):
        dev = res.results[c]["out"]        # [cap]; slot order == store order
        out[sels[c]] = dev[:len(sels[c])]
    return out


def kernel(**inputs):
    return _run(inputs, trace=bool(os.environ.get("BASS_TRACE")))

